# revision 1
# baseline (speedup 1.0000x reference)
"""GCN(2-layer) + single-step BiLSTM + MLP over a 100k-node/1.6M-edge graph,
distributed over 8 Trainium2 NeuronCores (Bass/Tile kernel).

Strategy (JIT-specialized to the input graph):
  - Destination nodes sharded across 8 cores (12500 each, padded to 16384
    slots so the global gather-source table has 4 clean 32768-row windows).
  - Per core, per GCN layer: pull-gather source rows with dma_gather (bf16,
    int16 window-local indices, 4 SWDGE queues in parallel), aggregate
    per-edge messages into PSUM with one-hot matmuls
    (S[e, dst_local] = dinv[src] * (dstloc[e] == iota), generated on DVE),
    into 28 PSUM columns (7 banks) per superblock of 3584 dsts.
  - Self-loops are regular edges; dinv[src] is folded into S; dinv[dst] is
    applied at PSUM drain.
  - Aggregate-first: h = relu((A_hat @ x) @ W + b). Layer boundary: on-chip
    transform + AllGather of the bf16 layer-2 gather source.
  - Tail (single-timestep BiLSTM + 2-layer MLP) is node-local, feature-major.

One SPMD NEFF for all 8 cores: loop bounds are structural maxima over the 8
cores' edge partitions; per-core data is padded to match (pad edges have
S rows = 0, so they contribute nothing).
"""
import sys
import os

sys.path.insert(0, '/opt/trn_rl_repo')

DEBUG_DUMPS = os.environ.get("KDBG", "0") == "1"

import numpy as np
import ml_dtypes

N = 100000
E_IN = 1600000
F = 128
NCORES = 8
NPC = 12500           # real dst nodes per core
SLOT = 16384          # padded node slots per core (4 * 4096)
SBN = 3584            # dsts per superblock (7 PSUM banks * 4 cols * 128)
NSB = 4               # superblocks per core: 3584*3 + 1748
NW = 4                # gather windows (consecutive core pairs)
WIN1 = 2 * NPC        # layer-1 window stride in node-id space (25000)
WIN2 = 2 * SLOT       # layer-2 window stride in gs-row space (32768)
GS_ROWS = NCORES * SLOT  # 131072
TILE = 128            # edges per aggregation matmul
CHUNK_TILES = 32      # tiles per dma_gather chunk (4096 idxs)
NQ = 4                # SWDGE queues

bf16 = ml_dtypes.bfloat16


def _blocks_of_sb(sb):
    if sb < 3:
        return 28
    return (NPC - 3 * SBN + 127) // 128  # 14 (last block has 84 real dsts)


NBLK = sum(_blocks_of_sb(sb) for sb in range(NSB))  # 98


def prep(edge_index):
    """Host-side graph preprocessing -> (meta, per_core)."""
    src_in = np.asarray(edge_index[0], dtype=np.int64)
    dst_in = np.asarray(edge_index[1], dtype=np.int64)
    loops = np.arange(N, dtype=np.int64)
    src = np.concatenate([src_in, loops])
    dst = np.concatenate([dst_in, loops])

    deg = (np.bincount(dst, minlength=N)).astype(np.float64)
    dinv = (np.maximum(deg, 1e-12) ** -0.5).astype(np.float32)

    gs_row_all = (src // NPC) * SLOT + (src % NPC)
    w_all = (src // NPC) // 2

    # ---- per-core sorted edge partitions + group sizes ----
    cores = []
    sizes = np.zeros((NCORES, NSB, NW, 28), dtype=np.int64)
    core_of = dst // NPC
    for c in range(NCORES):
        m = core_of == c
        s_c = src[m]
        dl = dst[m] - c * NPC
        sb = np.minimum(dl // SBN, NSB - 1)
        bi = (dl - sb * SBN) // 128
        w = w_all[m]
        order = np.lexsort((s_c, dl, bi, w, sb))
        d = {"s": s_c[order], "dl": dl[order], "sb": sb[order],
             "bi": bi[order], "w": w[order], "gs": gs_row_all[m][order]}
        cores.append(d)
        key = ((d["sb"] * NW + d["w"]) * 28 + d["bi"])
        sizes[c] = np.bincount(key, minlength=NSB * NW * 28).reshape(NSB, NW, 28)

    # ---- structural tile layout ----
    T = np.maximum(1, -(-sizes.max(axis=0) // TILE))  # [NSB, NW, 28]
    for sb in range(NSB):
        T[sb, :, _blocks_of_sb(sb):] = 0

    tbase = np.zeros((NSB, NW, 28), dtype=np.int64)
    Tsw = np.zeros((NSB, NW), dtype=np.int64)
    for sb in range(NSB):
        for w in range(NW):
            off = 0
            for b in range(_blocks_of_sb(sb)):
                tbase[sb, w, b] = off
                off += T[sb, w, b]
            Tsw[sb, w] = off

    chunks = {}
    for sb in range(NSB):
        for w in range(NW):
            lst, t0 = [], 0
            while t0 < Tsw[sb, w]:
                n = min(CHUNK_TILES, int(Tsw[sb, w]) - t0)
                lst.append((t0, n))
                t0 += n
            chunks[(sb, w)] = lst

    tinfo = {}
    for sb in range(NSB):
        for w in range(NW):
            nt = int(Tsw[sb, w])
            blk = np.zeros(nt, dtype=np.int64)
            first = np.zeros(nt, dtype=bool)
            last = np.zeros(nt, dtype=bool)
            for b in range(_blocks_of_sb(sb)):
                a, n = int(tbase[sb, w, b]), int(T[sb, w, b])
                blk[a:a + n] = b
                if w == 0:
                    first[a] = True
                if w == NW - 1:
                    last[a + n - 1] = True
            tinfo[(sb, w)] = (blk, first, last)

    swbase = np.zeros((NSB, NW), dtype=np.int64)
    off = 0
    for sb in range(NSB):
        for w in range(NW):
            swbase[sb, w] = off
            off += int(Tsw[sb, w]) * TILE
    total_pad = off  # padded edge count (same for all cores)

    # ---- per-core padded streams ----
    per_core = []
    for c in range(NCORES):
        d = cores[c]
        ne = len(d["s"])
        key = ((d["sb"] * NW + d["w"]) * 28 + d["bi"])
        cnt = np.bincount(key, minlength=NSB * NW * 28)
        grp_start = np.zeros(NSB * NW * 28 + 1, dtype=np.int64)
        grp_start[1:] = np.cumsum(cnt)
        rank = np.arange(ne) - grp_start[key]
        pos = (swbase[d["sb"], d["w"]]
               + tbase[d["sb"], d["w"], d["bi"]] * TILE + rank)

        idxg = np.zeros(total_pad, dtype=np.int16)
        dstloc = np.full(total_pad, 255.0, dtype=np.float32)
        dinvsrc = np.zeros(total_pad, dtype=np.float32)
        idxg[pos] = (d["gs"] - d["w"] * WIN2).astype(np.int16)
        dstloc[pos] = (d["dl"] % 128).astype(np.float32)
        dinvsrc[pos] = dinv[d["s"]]

        def pack_idx(idx):
            cols = []
            for sb in range(NSB):
                for w in range(NW):
                    base = int(swbase[sb, w])
                    for (t0, ntiles) in chunks[(sb, w)]:
                        a = base + t0 * TILE
                        blockv = idx[a:a + ntiles * TILE].reshape(-1, 16).T
                        cols.append(np.tile(blockv, (8, 1)))
            return np.ascontiguousarray(np.concatenate(cols, axis=1))

        dcols = np.zeros((128, 128), dtype=np.float32)
        gb = 0
        for sb in range(NSB):
            for b in range(_blocks_of_sb(sb)):
                lo = c * NPC + sb * SBN + b * 128
                hi = min(lo + 128, (c + 1) * NPC)
                if hi > lo:
                    dcols[:hi - lo, gb] = dinv[lo:hi]
                gb += 1

        dl_t = dstloc.reshape(-1, TILE).T          # [128, tot_tiles]
        f8 = ml_dtypes.float8_e4m3
        onehot = (dl_t[:, :, None] == np.arange(128, dtype=np.float32)[None, None, :])
        per_core.append({
            "idxg": pack_idx(idxg),
            "dstloc": np.ascontiguousarray(dl_t.astype(bf16)),
            "dinvsrc": np.ascontiguousarray(dinvsrc.reshape(-1, TILE).T.astype(bf16)),
            "s8": np.ascontiguousarray(onehot.astype(f8)),
            "dcols": dcols,
        })

    meta = {"T": T, "tbase": tbase, "Tsw": Tsw, "chunks": chunks,
            "tinfo": tinfo, "swbase": swbase, "total_pad": int(total_pad),
            "dinv": dinv}
    return meta, per_core


# ---------------------------------------------------------------------------
# numpy emulation of the aggregation pipeline (prep validation)
# ---------------------------------------------------------------------------

def emulate_layer(meta, per_core, table, layer, out_rows):
    """Emulate one GCN aggregation (no transform): returns agg [out_rows, F]
    where row c*NPC+dl holds dinv_d * sum_e dinv_s * table[src_row(e)].
    `table` is the gather source ([N,F] for layer 1, [GS_ROWS,F] for 2)."""
    chunks = meta["chunks"]
    tinfo = meta["tinfo"]
    win = WIN1 if layer == 1 else WIN2
    agg = np.zeros((out_rows, F), dtype=np.float32)
    for c in range(NCORES):
        pc = per_core[c]
        idxs = pc["idx1" if layer == 1 else "idx2"][:16, :]
        dl_t = pc["dstloc"].astype(np.float32)
        dv_t = pc["dinvsrc"].astype(np.float32)
        tile_cursor = 0
        col_cursor = 0
        acc = np.zeros((NSB, 28, 128, F), dtype=np.float32)
        for sb in range(NSB):
            for w in range(NW):
                blk, _, _ = tinfo[(sb, w)]
                for (t0, ntiles) in chunks[(sb, w)]:
                    n = ntiles * TILE
                    loc = idxs[:, col_cursor:col_cursor + n // 16].T.reshape(-1)
                    col_cursor += n // 16
                    rows = loc.astype(np.int64) + w * win
                    msgs = table[rows].reshape(ntiles, TILE, F).astype(np.float32)
                    for t in range(ntiles):
                        gt = tile_cursor + t
                        onehot = (dl_t[:, gt][:, None] == np.arange(128)[None, :])
                        S = onehot * dv_t[:, gt][:, None]
                        acc[sb, blk[t0 + t]] += S.T.astype(np.float32) @ msgs[t]
                    tile_cursor += ntiles
        for sb in range(NSB):
            for b in range(_blocks_of_sb(sb)):
                lo = c * NPC + sb * SBN + b * 128
                hi = min(lo + 128, (c + 1) * NPC)
                gb = sum(_blocks_of_sb(s) for s in range(sb)) + b
                dc = pc["dcols"][:hi - lo, gb]
                agg[lo:hi] = acc[sb, b][:hi - lo] * dc[:, None]
    return agg


# ---------------------------------------------------------------------------
# device program
# ---------------------------------------------------------------------------

NSLOT_USED = 3 * SBN + ((NPC - 3 * SBN + 127) // 128) * 128  # 12544 written slots


def build_nc(meta):
    import concourse.bacc as bacc
    import concourse.mybir as mybir
    import concourse.tile as tile
    from concourse.masks import make_identity

    dt = mybir.dt
    chunks = meta["chunks"]
    tinfo = meta["tinfo"]
    tot_tiles = meta["total_pad"] // TILE
    idx_cols_total = meta["total_pad"] // 16

    nc = bacc.Bacc("TRN2", target_bir_lowering=False, debug=False,
                   num_devices=NCORES, num_swdge_queues=NQ)

    # ---- I/O ----
    xshard = nc.dram_tensor("xshard", [NSLOT_USED, F], dt.bfloat16, kind="ExternalInput")
    idxg = nc.dram_tensor("idxg", [128, idx_cols_total], dt.int16, kind="ExternalInput")
    s_in = nc.dram_tensor("s8", [128, tot_tiles, 128], dt.float8e4, kind="ExternalInput")
    drow = nc.dram_tensor("drow", [128, NSLOT_USED], dt.float32, kind="ExternalInput")
    dcols_in = nc.dram_tensor("dcols", [128, 128], dt.float32, kind="ExternalInput")
    w1_in = nc.dram_tensor("w1", [128, 128], dt.bfloat16, kind="ExternalInput")
    w2_in = nc.dram_tensor("w2", [128, 128], dt.bfloat16, kind="ExternalInput")
    b1_in = nc.dram_tensor("b1c", [128, 1], dt.float32, kind="ExternalInput")
    b2_in = nc.dram_tensor("b2c", [128, 1], dt.float32, kind="ExternalInput")
    wihf_in = nc.dram_tensor("wihfT", [128, 512], dt.bfloat16, kind="ExternalInput")
    wihb_in = nc.dram_tensor("wihbT", [128, 512], dt.bfloat16, kind="ExternalInput")
    bsf_in = nc.dram_tensor("bsumf", [128, 4], dt.float32, kind="ExternalInput")
    bsb_in = nc.dram_tensor("bsumb", [128, 4], dt.float32, kind="ExternalInput")
    fc1_in = nc.dram_tensor("fc1", [128, 128], dt.bfloat16, kind="ExternalInput")
    fcb1_in = nc.dram_tensor("fcb1c", [64, 1], dt.float32, kind="ExternalInput")
    fc2_in = nc.dram_tensor("fc2", [64, 1], dt.bfloat16, kind="ExternalInput")
    fcb2_in = nc.dram_tensor("fcb2c", [1, 1], dt.float32, kind="ExternalInput")
    y_out = nc.dram_tensor("y", [NSLOT_USED, 1], dt.float32, kind="ExternalOutput")

    if DEBUG_DUMPS:
        dbg_gsin = nc.dram_tensor("dbg_gsin", [SLOT, F], dt.bfloat16, kind="ExternalOutput")
        dbg_h1 = nc.dram_tensor("dbg_h1", [128, NSLOT_USED], dt.bfloat16, kind="ExternalOutput")
        dbg_h2 = nc.dram_tensor("dbg_h2", [128, NSLOT_USED], dt.bfloat16, kind="ExternalOutput")
        dbg_st = nc.dram_tensor("dbg_st", [128, NSLOT_USED], dt.bfloat16, kind="ExternalOutput")

    gs2_in = nc.dram_tensor("gs2in", [SLOT, F], dt.bfloat16, kind="Internal")
    gs2 = nc.dram_tensor("gs2", [GS_ROWS, F], dt.bfloat16, kind="Internal",
                         addr_space="Shared")
    gs1_in = nc.dram_tensor("gs1in", [SLOT, F], dt.bfloat16, kind="Internal")
    gs1 = nc.dram_tensor("gs1", [GS_ROWS, F], dt.bfloat16, kind="Internal",
                         addr_space="Shared")

    qctr = [0]

    with tile.TileContext(nc) as tc:
        with tc.tile_pool(name="const", bufs=1) as cpool, \
             tc.tile_pool(name="idx", bufs=6) as ipool, \
             tc.tile_pool(name="msg", bufs=4) as mpool, \
             tc.tile_pool(name="sg", bufs=4) as spool, \
             tc.tile_pool(name="staged", bufs=2) as stpool, \
             tc.tile_pool(name="h1", bufs=2) as h1pool, \
             tc.tile_pool(name="h2", bufs=1) as h2pool, \
             tc.tile_pool(name="ndm", bufs=1) as ndmpool, \
             tc.tile_pool(name="tail", bufs=2) as tpool, \
             tc.tile_pool(name="psA", bufs=7, space="PSUM") as psA, \
             tc.tile_pool(name="psB", bufs=1, space="PSUM") as psB:

            # ---- constants ----
            def const_tile(shape, dtt, src_ap, cname):
                t = cpool.tile(shape, dtt, tag=cname, name=cname)
                nc.sync.dma_start(t[:], src_ap)
                return t

            dcols_t = const_tile([128, 128], dt.float32, dcols_in[:], "c_dcols")
            w1_t = const_tile([128, 128], dt.bfloat16, w1_in[:], "c_w1")
            w2_t = const_tile([128, 128], dt.bfloat16, w2_in[:], "c_w2")
            b1_t = const_tile([128, 1], dt.float32, b1_in[:], "c_b1")
            b2_t = const_tile([128, 1], dt.float32, b2_in[:], "c_b2")
            wihf_t = const_tile([128, 512], dt.bfloat16, wihf_in[:], "c_wihf")
            wihb_t = const_tile([128, 512], dt.bfloat16, wihb_in[:], "c_wihb")
            bsf_t = const_tile([128, 4], dt.float32, bsf_in[:], "c_bsf")
            bsb_t = const_tile([128, 4], dt.float32, bsb_in[:], "c_bsb")
            fc1_t = const_tile([128, 128], dt.bfloat16, fc1_in[:], "c_fc1")
            fcb1_t = const_tile([64, 1], dt.float32, fcb1_in[:], "c_fcb1")
            fc2_t = const_tile([64, 1], dt.bfloat16, fc2_in[:], "c_fc2")
            fcb2_t = const_tile([1, 1], dt.float32, fcb2_in[:], "c_fcb2")
            ident_t = cpool.tile([128, 128], dt.bfloat16)
            make_identity(nc, ident_t[:])

            h2T = h2pool.tile([128, NSLOT_USED], dt.bfloat16)

            # ---- prologue: gs1 = dinv * x (own shard), AllGather ----
            xsh = ndmpool.tile([128, 98, 128], dt.bfloat16, tag="xsh")
            nc.sync.dma_start(
                xsh[:],
                xshard[:].rearrange("(t p) f -> p t f", p=128))
            nc.vector.tensor_tensor(
                xsh[:], xsh[:],
                dcols_t[:, 0:98, None].to_broadcast([128, 98, 128]),
                mybir.AluOpType.mult)
            nc.sync.dma_start(
                gs1_in[0:98 * 128, :].rearrange("(t p) f -> p t f", p=128),
                xsh[:])
            nc.gpsimd.collective_compute(
                "AllGather", mybir.AluOpType.bypass,
                replica_groups=[list(range(NCORES))],
                ins=[gs1_in.ap()], outs=[gs1.ap()])

            # ---------------- one GCN layer ----------------
            def gcn_layer(layer_idx, src_tensor, idx_tensor, win_rows_total):
                icol = [0]
                tcol = [0]
                win = WIN2
                for sb in range(NSB):
                    ncols = _blocks_of_sb(sb)
                    nbank = (ncols + 3) // 4
                    aggs = [psA.tile([128, 512], dt.float32, tag="agg",
                                     name=f"agg_l{layer_idx}_sb{sb}_k{k}")
                            for k in range(nbank)]
                    for a in aggs:
                        nc.vector.memset(a[:], 0.0)
                    for w in range(NW):
                        blk, first, last = tinfo[(sb, w)]
                        lo = w * win
                        hi = min(lo + win, win_rows_total)
                        src_win = src_tensor[lo:hi, :]
                        for (t0, ntiles) in chunks[(sb, w)]:
                            nidx = ntiles * TILE
                            it = ipool.tile([128, CHUNK_TILES * 8], dt.int16, tag="idx")
                            nc.sync.dma_start(
                                it[:, :nidx // 16],
                                idx_tensor[:, icol[0]:icol[0] + nidx // 16])
                            icol[0] += nidx // 16
                            mt = mpool.tile([128, CHUNK_TILES, F], dt.bfloat16, tag="msg")
                            nc.gpsimd.dma_gather(
                                mt[:, :ntiles, :], src_win, it[:, :nidx // 16],
                                nidx, nidx, F, single_packet=False,
                                queue_num=qctr[0] % NQ)
                            qctr[0] += 1
                            st = spool.tile([128, CHUNK_TILES, 128], dt.float8e4, tag="S")
                            nc.scalar.dma_start(
                                st[:, :ntiles, :],
                                s_in[:, tcol[0]:tcol[0] + ntiles, :])
                            for t in range(ntiles):
                                b = int(blk[t0 + t])
                                bank, col = b // 4, b % 4
                                nc.tensor.matmul(
                                    aggs[bank][:, col * 128:(col + 1) * 128],
                                    lhsT=mt[:, t, :],
                                    rhs=st[:, t, :],
                                    start=False,
                                    stop=bool(last[t0 + t]),
                                    skip_group_check=True)
                            tcol[0] += ntiles
                    # ---- drain: staged = psum * dinv_dst (feature-major) ----
                    sb_base = sb * SBN
                    width = ncols * 128
                    dr = stpool.tile([128, 28 * 128], dt.float32, tag="drow", bufs=1)
                    nc.sync.dma_start(dr[:, :width],
                                      drow[:, sb_base:sb_base + width])
                    staged = stpool.tile([128, 28 * 128], dt.bfloat16, tag="staged")
                    for k in range(nbank):
                        wcols = min(4, ncols - k * 4) * 128
                        nc.vector.tensor_tensor(
                            staged[:, k * 512:k * 512 + wcols],
                            aggs[k][:, :wcols],
                            dr[:, k * 512:k * 512 + wcols],
                            mybir.AluOpType.mult)
                    if DEBUG_DUMPS and layer_idx == 1:
                        nc.sync.dma_start(dbg_st[:, sb_base:sb_base + width],
                                          staged[:, :width])
                    if layer_idx == 1:
                        # h1T = relu(W1.T @ staged + b1); gs2T = W2.T @ h1T
                        h1sb = h1pool.tile([128, 28 * 128], dt.bfloat16, tag="h1sb")
                        ch0 = 0
                        while ch0 < width:
                            cw = min(512, width - ch0)
                            ptx = psB.tile([128, 512], dt.float32, tag="tx")
                            nc.tensor.matmul(ptx[:, :cw], lhsT=w1_t[:],
                                             rhs=staged[:, ch0:ch0 + cw],
                                             start=True, stop=True)
                            nc.scalar.activation(
                                h1sb[:, ch0:ch0 + cw], ptx[:, :cw],
                                mybir.ActivationFunctionType.Relu, bias=b1_t[:])
                            ch0 += cw
                        ndm = ndmpool.tile([128, 28, 128], dt.bfloat16, tag="ndm")
                        ch0 = 0
                        while ch0 < width:
                            cw = min(512, width - ch0)
                            ptx = psB.tile([128, 512], dt.float32, tag="tx")
                            nc.tensor.matmul(ptx[:, :cw], lhsT=w2_t[:],
                                             rhs=h1sb[:, ch0:ch0 + cw],
                                             start=True, stop=True)
                            gsT = h1pool.tile([128, 512], dt.bfloat16, tag="gsT")
                            nc.vector.tensor_tensor(
                                gsT[:, :cw], ptx[:, :cw],
                                dr[:, ch0:ch0 + cw], mybir.AluOpType.mult)
                            for bb in range(cw // 128):
                                b = ch0 // 128 + bb
                                ptp = psB.tile([128, 512], dt.float32, tag="tx")
                                ptp_b = ptp[:].bitcast(dt.bfloat16)[:, :128]
                                nc.tensor.transpose(
                                    ptp_b, gsT[:, bb * 128:(bb + 1) * 128], ident_t[:])
                                nc.vector.tensor_copy(ndm[:, b, :], ptp_b)
                            ch0 += cw
                        nc.sync.dma_start(
                            gs2_in[sb * SBN: sb * SBN + width, :]
                            .rearrange("(c p) f -> p c f", p=128),
                            ndm[:, :ncols, :])
                        if DEBUG_DUMPS:
                            nc.sync.dma_start(
                                dbg_gsin[sb * SBN: sb * SBN + width, :]
                                .rearrange("(c p) f -> p c f", p=128),
                                ndm[:, :ncols, :])
                            nc.sync.dma_start(
                                dbg_h1[:, sb * SBN: sb * SBN + width],
                                h1sb[:, :width])
                    else:
                        nc.scalar.activation(
                            h2T[:, sb_base:sb_base + width],
                            staged[:, :width],
                            mybir.ActivationFunctionType.Relu, bias=b2_t[:])

            gcn_layer(1, gs1, idxg, GS_ROWS)

            nc.gpsimd.collective_compute(
                "AllGather", mybir.AluOpType.bypass,
                replica_groups=[list(range(NCORES))],
                ins=[gs2_in.ap()], outs=[gs2.ap()])

            gcn_layer(2, gs2, idxg, GS_ROWS)

            if DEBUG_DUMPS:
                nc.sync.dma_start(dbg_h2[:], h2T[:])

            # ---------------- tail: BiLSTM step + MLP ----------------
            ch0 = 0
            while ch0 < NSLOT_USED:
                cw = min(512, NSLOT_USED - ch0)
                h2c = h2T[:, ch0:ch0 + cw]
                hdir = []
                for (wih_t, bs_t) in ((wihf_t, bsf_t), (wihb_t, bsb_t)):
                    gates = {}
                    for jb, fn in ((0, "Sigmoid"), (2, "Tanh"), (3, "Sigmoid")):
                        pg = psA.tile([128, 512], dt.float32, tag="agg")
                        nc.tensor.matmul(pg[:, :cw],
                                         lhsT=wih_t[:, jb * 128:(jb + 1) * 128],
                                         rhs=h2c, start=True, stop=True)
                        gt = tpool.tile([128, 512], dt.bfloat16, tag=f"g{jb}")
                        nc.scalar.activation(
                            gt[:, :cw], pg[:, :cw],
                            getattr(mybir.ActivationFunctionType, fn),
                            bias=bs_t[:, jb:jb + 1])
                        gates[jb] = gt
                    c_t = tpool.tile([128, 512], dt.bfloat16, tag="c")
                    nc.vector.tensor_tensor(c_t[:, :cw], gates[0][:, :cw],
                                            gates[2][:, :cw], mybir.AluOpType.mult)
                    tc_t = tpool.tile([128, 512], dt.bfloat16, tag="tc")
                    nc.scalar.activation(tc_t[:, :cw], c_t[:, :cw],
                                         mybir.ActivationFunctionType.Tanh)
                    h_t = tpool.tile([128, 512], dt.bfloat16, tag=f"h{len(hdir)}")
                    nc.vector.tensor_tensor(h_t[:, :cw], gates[3][:, :cw],
                                            tc_t[:, :cw], mybir.AluOpType.mult)
                    hdir.append(h_t)
                py1 = psA.tile([128, 512], dt.float32, tag="agg")
                nc.tensor.matmul(py1[:64, :cw], lhsT=fc1_t[:, :64],
                                 rhs=hdir[0][:, :cw], start=True, stop=False)
                nc.tensor.matmul(py1[:64, :cw], lhsT=fc1_t[:, 64:],
                                 rhs=hdir[1][:, :cw], start=False, stop=True)
                y1_t = tpool.tile([64, 512], dt.bfloat16, tag="y1")
                nc.scalar.activation(y1_t[:, :cw], py1[:64, :cw],
                                     mybir.ActivationFunctionType.Relu,
                                     bias=fcb1_t[:])
                py2 = psA.tile([128, 512], dt.float32, tag="agg")
                nc.tensor.matmul(py2[:1, :cw], lhsT=fc2_t[:],
                                 rhs=y1_t[:, :cw], start=True, stop=True)
                ych = tpool.tile([1, 512], dt.float32, tag="ych")
                nc.vector.tensor_scalar_add(ych[0:1, :cw],
                                            py2[:1, :cw], fcb2_t[0:1, 0:1])
                nc.sync.dma_start(
                    y_out[ch0:ch0 + cw, :].rearrange("n o -> o n"),
                    ych[0:1, :cw])
                ch0 += cw

    nc.compile()
    return nc


_CACHE = {}


def _marshal(inputs, meta, per_core):
    x = np.asarray(inputs["x"], dtype=np.float32)
    dinv = meta["dinv"]
    in_common = {
        "w1": np.ascontiguousarray(np.asarray(inputs["W1"], np.float32).astype(bf16)),
        "w2": np.ascontiguousarray(np.asarray(inputs["W2"], np.float32).astype(bf16)),
        "b1c": np.ascontiguousarray(np.asarray(inputs["b1"], np.float32)[:, None]),
        "b2c": np.ascontiguousarray(np.asarray(inputs["b2"], np.float32)[:, None]),
        "wihfT": np.ascontiguousarray(
            np.asarray(inputs["Wih_f"], np.float32).T.astype(bf16)),
        "wihbT": np.ascontiguousarray(
            np.asarray(inputs["Wih_b"], np.float32).T.astype(bf16)),
        "bsumf": np.ascontiguousarray(
            (np.asarray(inputs["bih_f"], np.float32)
             + np.asarray(inputs["bhh_f"], np.float32)).reshape(4, 128).T),
        "bsumb": np.ascontiguousarray(
            (np.asarray(inputs["bih_b"], np.float32)
             + np.asarray(inputs["bhh_b"], np.float32)).reshape(4, 128).T),
        "fc1": np.ascontiguousarray(
            np.asarray(inputs["fcW1"], np.float32).astype(bf16)
            .reshape(2, 128, 64).transpose(1, 0, 2).reshape(128, 128)),
        "fcb1c": np.ascontiguousarray(np.asarray(inputs["fcb1"], np.float32)[:, None]),
        "fc2": np.ascontiguousarray(np.asarray(inputs["fcW2"], np.float32).astype(bf16)),
        "fcb2c": np.ascontiguousarray(
            np.asarray(inputs["fcb2"], np.float32).reshape(1, 1)),
    }
    in_maps = []
    for c in range(NCORES):
        pc = per_core[c]
        drow_c = np.zeros((1, NSLOT_USED), dtype=np.float32)
        drow_c[0, :NPC] = dinv[c * NPC:(c + 1) * NPC]
        drow_c = np.ascontiguousarray(np.tile(drow_c, (128, 1)))
        m = dict(in_common)
        m["drow"] = drow_c
        xs = np.zeros((NSLOT_USED, F), dtype=bf16)
        xs[:NPC] = x[c * NPC:(c + 1) * NPC].astype(bf16)
        m["xshard"] = xs
        m["idxg"] = pc["idxg"]
        m["s8"] = pc["s8"]
        m["dcols"] = np.ascontiguousarray(
            np.pad(pc["dcols"][:, :NBLK], ((0, 0), (0, 128 - NBLK))))
        in_maps.append(m)
    return in_maps


def get_compiled(edge_index):
    import hashlib
    edge_index = np.asarray(edge_index)
    key = hashlib.sha1(edge_index.tobytes()).hexdigest()
    if key not in _CACHE:
        meta, per_core = prep(edge_index)
        nc = build_nc(meta)
        _CACHE[key] = (meta, per_core, nc)
    return _CACHE[key]


def run(inputs, trace=False):
    from concourse.bass_utils import run_bass_kernel_spmd
    meta, per_core, nc = get_compiled(inputs["edge_index"])
    in_maps = _marshal(inputs, meta, per_core)
    res = run_bass_kernel_spmd(nc, in_maps, core_ids=list(range(NCORES)),
                               trace=trace)
    y = np.zeros((N, 1), dtype=np.float32)
    for c in range(NCORES):
        y[c * NPC:(c + 1) * NPC, 0] = np.asarray(res.results[c]["y"])[:NPC, 0]
    return y, res


def kernel(**inputs):
    y, _ = run(inputs, trace=False)
    return y

